# revision 9
# baseline (speedup 1.0000x reference)
"""Bass/Tile Trainium2 kernel for nn_BinaryLSTM (B=64,T=1024,D=128,H=512,C=2).

Strategy (data-parallel over batch, 8 cores x 8 sequences each):
  - All state transposed: hT/cT are [H(part), Bc(free)]; gates computed as
    zT [4H, Bc] via PE matmuls with Wh tiles stationary (bf16 -> fast weight
    load) and h chunks moving.
  - Per step: 64 (LDW+MM) pairs accumulate z = Wh^T h into one PSUM bank
    laid out [128, (16 m-tiles, 8 batch)]; m-tile order is gate-interleaved
    (i,f,o,g per h-chunk) so the per-h-chunk pointwise chain (sigmoid/tanh,
    cell update) starts before the step's matmuls finish.
  - x @ Wx + b precomputed per 64-step chunk on the PE (moving = pre-
    transposed x from host), stored bf16 in SBUF, added via DVE.
  - Output head: C=2 softmax == sigmoid of logit difference; probabilities
    computed on device, NLL/cost reduced on host from them (trivial glue).
"""

import numpy as np
import ml_dtypes

B, T, D, H, C = 64, 1024, 128, 512, 2
NCORES = 8
BC = B // NCORES          # sequences per core
S = 64                    # timesteps per chunk
NCHUNK = T // S           # 16
BODY_CHUNKS = 2           # chunks per For_i body (static ping-pong period)
NITER = NCHUNK // BODY_CHUNKS
TPAD = T + BODY_CHUNKS * S  # x time-padding so the last body's prefetch reads zeros
NM = 16                   # gate m-tiles (4 h-chunks x {i,f,o,g})
KC = 4                    # contraction chunks of H

# weight-column base per gate in keras order [i,f,g,o] -> our order (i,f,o,g)
_GATE_BASE = (0, H, 3 * H, 2 * H)  # i, f, o, g


def _build_nc():
    import concourse.bass as bass
    import concourse.bacc as bacc
    import concourse.tile as tile
    import concourse.mybir as mybir

    BF16 = mybir.dt.bfloat16
    F32 = mybir.dt.float32
    AF = mybir.ActivationFunctionType

    nc = bacc.Bacc("TRN2", target_bir_lowering=False, debug=False,
                   num_devices=NCORES)

    whp_d = nc.declare_dram_parameter("whp", [128, KC, NM, 128], BF16, isOutput=False)
    wxp_d = nc.declare_dram_parameter("wxp", [128, NM, 128], BF16, isOutput=False)
    bt_d = nc.declare_dram_parameter("bt", [128, NM], F32, isOutput=False)
    xt_d = nc.declare_dram_parameter("xt", [128, TPAD, BC], BF16, isOutput=False)
    wo_d = nc.declare_dram_parameter("wo", [128, KC, C], BF16, isOutput=False)
    bo2_d = nc.declare_dram_parameter("bo2", [128, C], F32, isOutput=False)
    pred_d = nc.declare_dram_parameter("pred", [128, T * BC // 128, C], F32, isOutput=True)

    NG = T * BC // 128  # 64 head row-tiles

    with tile.TileContext(nc) as tc:
        with (
            tc.tile_pool(name="const", bufs=1) as const,
            tc.tile_pool(name="state", bufs=1) as state,
            tc.tile_pool(name="work", bufs=8) as work,
            tc.tile_pool(name="zps", bufs=2, space="PSUM") as zps,
            tc.tile_pool(name="xps", bufs=2, space="PSUM") as xps,
            tc.tile_pool(name="hps", bufs=2, space="PSUM") as hps,
        ):
            whp = const.tile([128, KC, NM, 128], BF16)
            wxp = const.tile([128, NM, 128], BF16)
            bt = const.tile([128, NM], F32)
            xt = const.tile([128, TPAD, BC], BF16)
            wo = const.tile([128, KC, C], BF16)
            bo2 = const.tile([128, C], F32)
            nc.sync.dma_start(whp[:], whp_d[:])
            nc.sync.dma_start(wxp[:], wxp_d[:])
            nc.sync.dma_start(bt[:], bt_d[:])
            nc.sync.dma_start(xt[:], xt_d[:])
            nc.sync.dma_start(wo[:], wo_d[:])
            nc.sync.dma_start(bo2[:], bo2_d[:])

            bodiff = state.tile([128, 1], F32)
            nbodiff = state.tile([128, 1], F32)
            nc.vector.tensor_sub(bodiff[:], bo2[:, 1:2], bo2[:, 0:1])
            nc.vector.tensor_scalar_mul(nbodiff[:], bodiff[:], -1.0)

            cT = state.tile([128, KC * BC], F32)          # cell state [h, b]
            hsb = [state.tile([128, KC, S + 1, BC], BF16, tag=f"hs{i}",
                              name=f"hs{i}") for i in range(2)]
            xzb = [state.tile([128, NM, S, BC], BF16, tag=f"xz{i}",
                              name=f"xz{i}") for i in range(2)]
            pred_sb = state.tile([128, NG, C], F32)

            nc.vector.memset(cT[:], 0.0)
            # slot S of buffer 1 acts as "h before chunk 0" = zeros
            nc.vector.memset(hsb[1][:, :, S, :], 0.0)

            def xz_compute(dst, t0_expr):
                """xz[m] = Wx_m^T @ xT[:, t0:t0+S, :] + b_m  -> bf16 SBUF."""
                for m in range(NM):
                    xp = xps.tile([128, S * BC], F32, tag="xp")
                    nc.tensor.matmul(
                        xp[:], wxp[:, m, :], xt[:, bass.ds(t0_expr, S), :],
                        start=True, stop=True)
                    if m % 2 == 0:
                        nc.scalar.activation(
                            dst[:, m, :, :], xp[:], AF.Identity,
                            bias=bt[:, m:m + 1])
                    else:
                        nc.vector.tensor_scalar_add(
                            dst[:, m, :, :], xp[:], bt[:, m:m + 1])

            def step(p, t):
                """One LSTM timestep: hsb[p] slot t -> slot t+1.

                All 64 matmuls accumulate into one PSUM bank (single
                accumulation group: start on the first MM, stop on the
                last); the pointwise runs once, full width, to minimize
                chain op count and per-op fixed costs."""
                zp = zps.tile([128, NM * BC], F32, tag="zp")
                mv = [hsb[p][:, k, t, :] for k in range(KC)]
                for k in range(KC):
                    for m in range(NM):
                        nc.tensor.matmul(
                            zp[:, m * BC:(m + 1) * BC], whp[:, k, m, :], mv[k],
                            start=(k == 0 and m == 0),
                            stop=(k == KC - 1 and m == NM - 1),
                            skip_group_check=True)
                # zs layout [128, (4 groups), (4 gates i,f,o,g), BC]
                zs = work.tile([128, 4, 4 * BC], F32, tag="zs")
                nc.vector.tensor_add(zs[:], zp[:], xzb[p][:, :, t, :])
                sg = work.tile([128, 4, 3 * BC], F32, tag="sg")
                nc.scalar.activation(sg[:], zs[:, :, 0:3 * BC], AF.Sigmoid)
                gt = work.tile([128, 4, BC], F32, tag="gt")
                nc.scalar.activation(gt[:], zs[:, :, 3 * BC:4 * BC], AF.Tanh)
                u = work.tile([128, 4 * BC], F32, tag="u")
                nc.vector.tensor_mul(u[:], sg[:, :, 0:BC], gt[:])
                cf = work.tile([128, 4 * BC], F32, tag="cf")
                nc.vector.tensor_mul(cf[:], sg[:, :, BC:2 * BC], cT[:])
                nc.vector.tensor_add(cT[:], u[:], cf[:])
                tc2 = work.tile([128, 4 * BC], F32, tag="tc2")
                nc.scalar.activation(tc2[:], cT[:], AF.Tanh)
                nc.vector.tensor_mul(
                    hsb[p][:, :, t + 1, :], sg[:, :, 2 * BC:3 * BC], tc2[:])

            def head(p, gbase_expr):
                """logit diff + sigmoid for chunk in hsb[p]; writes pred_sb."""
                rows_per_mt = 128 // BC  # 16 timesteps per row-tile
                for mt in range(S * BC // 128):  # 4
                    lp = hps.tile([128, C], F32, tag="lp")
                    t0 = 1 + rows_per_mt * mt
                    for j in range(KC):
                        nc.tensor.matmul(
                            lp[:], hsb[p][:, j, t0:t0 + rows_per_mt, :],
                            wo[:, j, :], start=(j == 0), stop=(j == KC - 1))
                    l0 = work.tile([128, 1], F32, tag="l0")
                    nc.scalar.copy(l0[:], lp[:, 0:1])
                    dlt = work.tile([128, 1], F32, tag="dlt")
                    nc.vector.tensor_sub(dlt[:], lp[:, 1:2], l0[:])
                    gsl = bass.ds(gbase_expr + mt, 1)
                    nc.scalar.activation(
                        pred_sb[:, gsl, 1:2], dlt[:], AF.Sigmoid,
                        bias=bodiff[:, 0:1])
                    nc.scalar.activation(
                        pred_sb[:, gsl, 0:1], dlt[:], AF.Sigmoid,
                        bias=nbodiff[:, 0:1], scale=-1.0)

            # prologue: xz for chunks 0,1
            xz_compute(xzb[0], 0)
            xz_compute(xzb[1], S)

            with tc.For_i(0, NITER) as it:
                for cc in range(BODY_CHUNKS):
                    p = cc  # chunk (2*it + cc) uses buffer cc
                    # carry h across chunks: slot 0 <- other buffer's slot S
                    nc.vector.tensor_copy(
                        hsb[p][:, :, 0, :], hsb[1 - p][:, :, S, :])
                    for t in range(S):
                        step(p, t)
                    head(p, it * (4 * BODY_CHUNKS) + cc * 4)
                    # prefetch xz for chunk (2*it + cc + BODY_CHUNKS)
                    xz_compute(xzb[p], it * (S * BODY_CHUNKS) + (cc + BODY_CHUNKS) * S)

            nc.sync.dma_start(pred_d[:], pred_sb[:])

    nc.compile()
    return nc


_NC_CACHE = None
TRACE = False  # test harness sets True (requires NTFF hook installed)
LAST_EXEC_NS = None


def _get_nc():
    global _NC_CACHE
    if _NC_CACHE is None:
        _NC_CACHE = _build_nc()
    return _NC_CACHE


def _wcol(m):
    """weight column slice for m-tile m = 4*j + gi (order i,f,o,g)."""
    j, gi = divmod(m, 4)
    base = _GATE_BASE[gi] + 128 * j
    return slice(base, base + 128)


def kernel(x, labels, Wx, Wh, b, Wo, bo):
    x = np.asarray(x, dtype=np.float32)
    labels = np.asarray(labels)
    Wx = np.asarray(Wx, dtype=np.float32)
    Wh = np.asarray(Wh, dtype=np.float32)
    b = np.asarray(b, dtype=np.float32)
    Wo = np.asarray(Wo, dtype=np.float32)
    bo = np.asarray(bo, dtype=np.float32)

    bf = ml_dtypes.bfloat16

    # --- host-side weight prep (shared across cores) ---
    whp = np.empty((128, KC, NM, 128), dtype=bf)
    wxp = np.empty((128, NM, 128), dtype=bf)
    bt = np.empty((128, NM), dtype=np.float32)
    for m in range(NM):
        cols = _wcol(m)
        for k in range(KC):
            whp[:, k, m, :] = Wh[k * 128:(k + 1) * 128, cols].astype(bf)
        wxp[:, m, :] = Wx[:, cols].astype(bf)
        bt[:, m] = b[cols]
    wo = np.empty((128, KC, C), dtype=bf)
    for j in range(KC):
        wo[:, j, :] = Wo[j * 128:(j + 1) * 128, :].astype(bf)
    bo2 = np.tile(bo.reshape(1, C), (128, 1)).astype(np.float32)

    nc = _get_nc()
    from concourse.bass_utils import run_bass_kernel_spmd

    core_ids = list(range(NCORES))
    in_maps = []
    for ci in core_ids:
        xs = x[ci * BC:(ci + 1) * BC]          # [BC, T, D]
        xtp = np.zeros((128, TPAD, BC), dtype=bf)
        xtp[:, :T, :] = np.ascontiguousarray(xs.transpose(2, 1, 0)).astype(bf)
        in_maps.append({
            "whp": whp, "wxp": wxp, "bt": bt, "xt": xtp,
            "wo": wo, "bo2": bo2,
        })
    global LAST_EXEC_NS
    res = run_bass_kernel_spmd(nc, in_maps, core_ids, trace=TRACE)
    LAST_EXEC_NS = res.exec_time_ns

    # --- unshard: pred_sb [128, 64, 2] -> [BC, T, 2] per core ---
    pred = np.empty((B, T, C), dtype=np.float32)
    for ci in core_ids:
        arr = res.results[ci]["pred"]            # [128, 64, 2]
        a = arr.reshape(128, NCHUNK, 4, C)       # [p, chunk, mt, c]
        a = a.transpose(1, 2, 0, 3).reshape(NCHUNK, 512, C)  # rows=(t%16..,b)
        a = a.reshape(NCHUNK, S, BC, C)          # [chunk, t_in_chunk, b, c]
        a = a.transpose(2, 0, 1, 3).reshape(BC, T, C)
        pred[ci * BC:(ci + 1) * BC] = a

    py = np.take_along_axis(pred, labels.astype(np.int64)[..., None], axis=-1)[..., 0]
    cost = np.float32(-np.log(np.maximum(py.astype(np.float64), 1e-30)).mean())
    return pred, cost


if __name__ == "__main__":
    rng = np.random.default_rng(0)
    inputs = {
        "x": rng.standard_normal((B, T, D), dtype=np.float32),
        "labels": rng.integers(0, C, size=(B, T)).astype(np.int32),
        "Wx": (rng.standard_normal((D, 4 * H)) / np.sqrt(D)).astype(np.float32),
        "Wh": (rng.standard_normal((H, 4 * H)) / np.sqrt(H)).astype(np.float32),
        "b": np.zeros(4 * H, np.float32),
        "Wo": (rng.standard_normal((H, C)) / np.sqrt(H)).astype(np.float32),
        "bo": np.zeros(C, np.float32),
    }
    p, cst = kernel(**inputs)
    print("pred", p.shape, p.dtype, "cost", cst)


# revision 10
# speedup vs baseline: 1.1545x; 1.1545x over previous
"""Bass/Tile Trainium2 kernel for nn_BinaryLSTM (B=64,T=1024,D=128,H=512,C=2).

Strategy (data-parallel over batch, 8 cores x 8 sequences each):
  - All state transposed: hT/cT are [H(part), Bc(free)]; gates computed as
    zT [4H, Bc] via PE matmuls with Wh tiles stationary (bf16 -> fast weight
    load) and h chunks moving.
  - Per step: 64 (LDW+MM) pairs accumulate z = Wh^T h into one PSUM bank
    laid out [128, (16 m-tiles, 8 batch)]; m-tile order is gate-interleaved
    (i,f,o,g per h-chunk) so the per-h-chunk pointwise chain (sigmoid/tanh,
    cell update) starts before the step's matmuls finish.
  - x @ Wx + b precomputed per 64-step chunk on the PE (moving = pre-
    transposed x from host), stored bf16 in SBUF, added via DVE.
  - Output head: C=2 softmax == sigmoid of logit difference; probabilities
    computed on device, NLL/cost reduced on host from them (trivial glue).
"""

import numpy as np
import ml_dtypes

B, T, D, H, C = 64, 1024, 128, 512, 2
NCORES = 8
BC = B // NCORES          # sequences per core
S = 64                    # timesteps per chunk
NCHUNK = T // S           # 16
BODY_CHUNKS = 2           # chunks per For_i body (static ping-pong period)
NITER = NCHUNK // BODY_CHUNKS
TPAD = T + BODY_CHUNKS * S  # x time-padding so the last body's prefetch reads zeros
NM = 16                   # gate m-tiles (4 h-chunks x {i,f,o,g})
KC = 4                    # contraction chunks of H

# weight-column base per gate in keras order [i,f,g,o] -> our order (i,f,o,g)
_GATE_BASE = (0, H, 3 * H, 2 * H)  # i, f, o, g


def _build_nc():
    import concourse.bass as bass
    import concourse.bacc as bacc
    import concourse.tile as tile
    import concourse.mybir as mybir

    BF16 = mybir.dt.bfloat16
    F32 = mybir.dt.float32
    AF = mybir.ActivationFunctionType

    nc = bacc.Bacc("TRN2", target_bir_lowering=False, debug=False,
                   num_devices=NCORES)

    whp_d = nc.declare_dram_parameter("whp", [128, KC, NM, 128], BF16, isOutput=False)
    wxp_d = nc.declare_dram_parameter("wxp", [128, NM, 128], BF16, isOutput=False)
    bt_d = nc.declare_dram_parameter("bt", [128, NM], F32, isOutput=False)
    xt_d = nc.declare_dram_parameter("xt", [128, TPAD, BC], BF16, isOutput=False)
    wo_d = nc.declare_dram_parameter("wo", [128, KC, C], BF16, isOutput=False)
    bo2_d = nc.declare_dram_parameter("bo2", [128, C], F32, isOutput=False)
    pred_d = nc.declare_dram_parameter("pred", [128, T * BC // 128, C], F32, isOutput=True)

    NG = T * BC // 128  # 64 head row-tiles

    with tile.TileContext(nc) as tc:
        with (
            tc.tile_pool(name="const", bufs=1) as const,
            tc.tile_pool(name="state", bufs=1) as state,
            tc.tile_pool(name="work", bufs=6) as work,
            tc.tile_pool(name="zps", bufs=2, space="PSUM") as zps,
            tc.tile_pool(name="xps", bufs=2, space="PSUM") as xps,
            tc.tile_pool(name="hps", bufs=2, space="PSUM") as hps,
        ):
            whp = const.tile([128, KC, NM, 128], BF16)
            wxp = const.tile([128, NM, 128], BF16)
            bt = const.tile([128, NM], F32)
            xt = const.tile([128, TPAD, BC], BF16)
            wo = const.tile([128, KC, C], BF16)
            bo2 = const.tile([128, C], F32)
            nc.sync.dma_start(whp[:], whp_d[:])
            nc.sync.dma_start(wxp[:], wxp_d[:])
            nc.sync.dma_start(bt[:], bt_d[:])
            nc.sync.dma_start(xt[:], xt_d[:])
            nc.sync.dma_start(wo[:], wo_d[:])
            nc.sync.dma_start(bo2[:], bo2_d[:])

            bodiff = state.tile([128, 1], F32)
            nbodiff = state.tile([128, 1], F32)
            nc.vector.tensor_sub(bodiff[:], bo2[:, 1:2], bo2[:, 0:1])
            nc.vector.tensor_scalar_mul(nbodiff[:], bodiff[:], -1.0)

            cT = state.tile([128, KC * BC], F32)          # cell state [h, b]
            hsb = [state.tile([128, KC, S + 1, BC], BF16, tag=f"hs{i}",
                              name=f"hs{i}") for i in range(2)]
            xzb = [state.tile([128, NM, S, BC], BF16, tag=f"xz{i}",
                              name=f"xz{i}") for i in range(2)]
            pred_sb = state.tile([128, NG, C], F32)

            nc.vector.memset(cT[:], 0.0)
            # slot S of buffer 1 acts as "h before chunk 0" = zeros
            nc.vector.memset(hsb[1][:, :, S, :], 0.0)

            def xz_compute(dst, t0_expr):
                """xz[m] = Wx_m^T @ xT[:, t0:t0+S, :] + b_m  -> bf16 SBUF."""
                for m in range(NM):
                    xp = xps.tile([128, S * BC], F32, tag="xp")
                    nc.tensor.matmul(
                        xp[:], wxp[:, m, :], xt[:, bass.ds(t0_expr, S), :],
                        start=True, stop=True)
                    if m % 2 == 0:
                        nc.scalar.activation(
                            dst[:, m, :, :], xp[:], AF.Identity,
                            bias=bt[:, m:m + 1])
                    else:
                        nc.vector.tensor_scalar_add(
                            dst[:, m, :, :], xp[:], bt[:, m:m + 1])

            def step(p, t):
                """One LSTM timestep: hsb[p] slot t -> slot t+1.

                z split across two PSUM banks (m-tiles 0-7 / 8-15) so the
                pointwise for groups {0,1} starts while the PE still works
                on groups {2,3}; pointwise merged per bank to halve the
                ACT/DVE per-op fixed costs."""
                zpA = zps.tile([128, 8 * BC], F32, tag="zpA")
                zpB = zps.tile([128, 8 * BC], F32, tag="zpB")
                zts = (zpA, zpB)
                mv = [hsb[p][:, k, t, :] for k in range(KC)]

                def mm(m, k, start, stop):
                    zt = zts[m // 8]
                    off = (m % 8) * BC
                    nc.tensor.matmul(
                        zt[:, off:off + BC], whp[:, k, m, :], mv[k],
                        start=start, stop=stop, skip_group_check=True)

                # Bank-blocked order: all four k-passes of bank A
                # (m 0-7, h-chunks 0/1) first, so its pointwise chain runs
                # under bank B's matmuls and the next step's bank-A k0/k1
                # find their h chunks ready.
                for pair in range(2):
                    for k in range(4):
                        for m in range(pair * 8, pair * 8 + 8):
                            mm(m, k, start=(k == 0 and m % 8 == 0),
                               stop=(k == 3 and m % 8 == 7))
                    zt = zts[pair]
                    # zs layout [128, (2 groups), (4 gates i,f,o,g), BC]
                    zs = work.tile([128, 2, 4 * BC], F32, tag="zs")
                    nc.vector.tensor_add(
                        zs[:], zt[:],
                        xzb[p][:, 8 * pair:8 * pair + 8, t, :])
                    sg = work.tile([128, 2, 3 * BC], F32, tag="sg")
                    nc.scalar.activation(sg[:], zs[:, :, 0:3 * BC], AF.Sigmoid)
                    gt = work.tile([128, 2, BC], F32, tag="gt")
                    nc.scalar.activation(gt[:], zs[:, :, 3 * BC:4 * BC], AF.Tanh)
                    u = work.tile([128, 2 * BC], F32, tag="u")
                    nc.vector.tensor_mul(u[:], sg[:, :, 0:BC], gt[:])
                    cf = work.tile([128, 2 * BC], F32, tag="cf")
                    cslice = cT[:, pair * 2 * BC:(pair + 1) * 2 * BC]
                    nc.vector.tensor_mul(cf[:], sg[:, :, BC:2 * BC], cslice)
                    nc.vector.tensor_add(cslice, u[:], cf[:])
                    tc2 = work.tile([128, 2 * BC], F32, tag="tc2")
                    nc.scalar.activation(tc2[:], cslice, AF.Tanh)
                    nc.vector.tensor_mul(
                        hsb[p][:, 2 * pair:2 * pair + 2, t + 1, :],
                        sg[:, :, 2 * BC:3 * BC], tc2[:])

            def head(p, gbase_expr):
                """logit diff + sigmoid for chunk in hsb[p]; writes pred_sb."""
                rows_per_mt = 128 // BC  # 16 timesteps per row-tile
                for mt in range(S * BC // 128):  # 4
                    lp = hps.tile([128, C], F32, tag="lp")
                    t0 = 1 + rows_per_mt * mt
                    for j in range(KC):
                        nc.tensor.matmul(
                            lp[:], hsb[p][:, j, t0:t0 + rows_per_mt, :],
                            wo[:, j, :], start=(j == 0), stop=(j == KC - 1))
                    l0 = work.tile([128, 1], F32, tag="l0")
                    nc.scalar.copy(l0[:], lp[:, 0:1])
                    dlt = work.tile([128, 1], F32, tag="dlt")
                    nc.vector.tensor_sub(dlt[:], lp[:, 1:2], l0[:])
                    gsl = bass.ds(gbase_expr + mt, 1)
                    nc.scalar.activation(
                        pred_sb[:, gsl, 1:2], dlt[:], AF.Sigmoid,
                        bias=bodiff[:, 0:1])
                    nc.scalar.activation(
                        pred_sb[:, gsl, 0:1], dlt[:], AF.Sigmoid,
                        bias=nbodiff[:, 0:1], scale=-1.0)

            # prologue: xz for chunks 0,1
            xz_compute(xzb[0], 0)
            xz_compute(xzb[1], S)

            with tc.For_i(0, NITER) as it:
                for cc in range(BODY_CHUNKS):
                    p = cc  # chunk (2*it + cc) uses buffer cc
                    # carry h across chunks: slot 0 <- other buffer's slot S
                    nc.vector.tensor_copy(
                        hsb[p][:, :, 0, :], hsb[1 - p][:, :, S, :])
                    for t in range(S):
                        step(p, t)
                    head(p, it * (4 * BODY_CHUNKS) + cc * 4)
                    # prefetch xz for chunk (2*it + cc + BODY_CHUNKS)
                    xz_compute(xzb[p], it * (S * BODY_CHUNKS) + (cc + BODY_CHUNKS) * S)

            nc.sync.dma_start(pred_d[:], pred_sb[:])

    nc.compile()
    return nc


_NC_CACHE = None
TRACE = False  # test harness sets True (requires NTFF hook installed)
LAST_EXEC_NS = None


def _get_nc():
    global _NC_CACHE
    if _NC_CACHE is None:
        _NC_CACHE = _build_nc()
    return _NC_CACHE


def _wcol(m):
    """weight column slice for m-tile m = 4*j + gi (order i,f,o,g)."""
    j, gi = divmod(m, 4)
    base = _GATE_BASE[gi] + 128 * j
    return slice(base, base + 128)


def kernel(x, labels, Wx, Wh, b, Wo, bo):
    x = np.asarray(x, dtype=np.float32)
    labels = np.asarray(labels)
    Wx = np.asarray(Wx, dtype=np.float32)
    Wh = np.asarray(Wh, dtype=np.float32)
    b = np.asarray(b, dtype=np.float32)
    Wo = np.asarray(Wo, dtype=np.float32)
    bo = np.asarray(bo, dtype=np.float32)

    bf = ml_dtypes.bfloat16

    # --- host-side weight prep (shared across cores) ---
    whp = np.empty((128, KC, NM, 128), dtype=bf)
    wxp = np.empty((128, NM, 128), dtype=bf)
    bt = np.empty((128, NM), dtype=np.float32)
    for m in range(NM):
        cols = _wcol(m)
        for k in range(KC):
            whp[:, k, m, :] = Wh[k * 128:(k + 1) * 128, cols].astype(bf)
        wxp[:, m, :] = Wx[:, cols].astype(bf)
        bt[:, m] = b[cols]
    wo = np.empty((128, KC, C), dtype=bf)
    for j in range(KC):
        wo[:, j, :] = Wo[j * 128:(j + 1) * 128, :].astype(bf)
    bo2 = np.tile(bo.reshape(1, C), (128, 1)).astype(np.float32)

    nc = _get_nc()
    from concourse.bass_utils import run_bass_kernel_spmd

    core_ids = list(range(NCORES))
    in_maps = []
    for ci in core_ids:
        xs = x[ci * BC:(ci + 1) * BC]          # [BC, T, D]
        xtp = np.zeros((128, TPAD, BC), dtype=bf)
        xtp[:, :T, :] = np.ascontiguousarray(xs.transpose(2, 1, 0)).astype(bf)
        in_maps.append({
            "whp": whp, "wxp": wxp, "bt": bt, "xt": xtp,
            "wo": wo, "bo2": bo2,
        })
    global LAST_EXEC_NS
    res = run_bass_kernel_spmd(nc, in_maps, core_ids, trace=TRACE)
    LAST_EXEC_NS = res.exec_time_ns

    # --- unshard: pred_sb [128, 64, 2] -> [BC, T, 2] per core ---
    pred = np.empty((B, T, C), dtype=np.float32)
    for ci in core_ids:
        arr = res.results[ci]["pred"]            # [128, 64, 2]
        a = arr.reshape(128, NCHUNK, 4, C)       # [p, chunk, mt, c]
        a = a.transpose(1, 2, 0, 3).reshape(NCHUNK, 512, C)  # rows=(t%16..,b)
        a = a.reshape(NCHUNK, S, BC, C)          # [chunk, t_in_chunk, b, c]
        a = a.transpose(2, 0, 1, 3).reshape(BC, T, C)
        pred[ci * BC:(ci + 1) * BC] = a

    py = np.take_along_axis(pred, labels.astype(np.int64)[..., None], axis=-1)[..., 0]
    cost = np.float32(-np.log(np.maximum(py.astype(np.float64), 1e-30)).mean())
    return pred, cost


if __name__ == "__main__":
    rng = np.random.default_rng(0)
    inputs = {
        "x": rng.standard_normal((B, T, D), dtype=np.float32),
        "labels": rng.integers(0, C, size=(B, T)).astype(np.int32),
        "Wx": (rng.standard_normal((D, 4 * H)) / np.sqrt(D)).astype(np.float32),
        "Wh": (rng.standard_normal((H, 4 * H)) / np.sqrt(H)).astype(np.float32),
        "b": np.zeros(4 * H, np.float32),
        "Wo": (rng.standard_normal((H, C)) / np.sqrt(H)).astype(np.float32),
        "bo": np.zeros(C, np.float32),
    }
    p, cst = kernel(**inputs)
    print("pred", p.shape, p.dtype, "cost", cst)
